# revision 36
# baseline (speedup 1.0000x reference)
"""BiDAF attention-flow kernel for Trainium2 (8 NeuronCores, data-parallel).

Self-contained: hardcodes shapes B,C,Q,H2 = 64,512,64,256; n_labels=2.
kernel(**inputs) takes full unsharded inputs, shards batch over 8 cores,
runs one SPMD Bass/Tile kernel, gathers [8,2] per core -> [64,2].

Per-core math (8 examples, bf16 compute, fp32 accumulation):
  S = c @ diag(w_m) @ q^T + (c@w_c)[:,None] + (q@w_q)[None,:]
    - the c@w_c term folds into the matmul rhs (rhs = w_m*q^T + w_c),
    - the q@w_q term rides in via a K=1 all-ones broadcast matmul.
  P = exp(S) unstabilized (|S| is O(1) for this distribution), so
  row-softmax needs only row-sums, and b_att = softmax(max_j S) is just
  Pmax/sum(Pmax) with Pmax = max_j P  (exp is monotone).
  All transposes (c^T, q^T, P_norm^T, q2c^T) go through the PE transpose
  path (DMA xbar transposes measured ~1.2us each, serial on the sync
  queue).  Max-pools over context run in the d-major layout as 2x-mode
  tensor_tensor max folds plus a short 1x reduce; the c*q2c piece uses
  max(q2c*cmax, q2c*cmin) so it needs no extra pass over the data.
"""

import os
import sys

for _p in ("/opt/trn_rl_repo", "/opt/pypackages"):
    if os.path.isdir(_p) and _p not in sys.path:
        sys.path.insert(0, _p)

import numpy as np

import concourse.bass as bass
import concourse.bacc as bacc
import concourse.tile as tile
import concourse.mybir as mybir
from concourse.bass_utils import run_bass_kernel_spmd
from concourse.masks import make_identity
from concourse.tile_rust import add_dep_helper

F32 = mybir.dt.float32
BF16 = mybir.dt.bfloat16
AX = mybir.AxisListType
OP = mybir.AluOpType
AF = mybir.ActivationFunctionType

DEBUG_DUMP = False

N_CORES = 8
B, C, Q, H2 = 64, 512, 64, 256
NL = 2
EX = B // N_CORES          # examples per core = 8
CH = C // 128              # context chunks of 128 = 4
DH = H2 // 128             # feature chunks of 128 = 2
NK = 4 * DH                # final feature chunks (4 pieces x DH) = 8


def _body(tc, ctx, fd, fq, wsim, wlab, blab, out):
    nc = tc.nc

    consts = ctx.enter_context(tc.tile_pool(name="consts", bufs=1))
    bigbuf = ctx.enter_context(tc.tile_pool(name="bigbuf", bufs=1))
    qm_pool = ctx.enter_context(tc.tile_pool(name="qm", bufs=3))
    qw_pool = ctx.enter_context(tc.tile_pool(name="qw", bufs=3))
    p_pool = ctx.enter_context(tc.tile_pool(name="p", bufs=2))
    pn_pool = ctx.enter_context(tc.tile_pool(name="pn", bufs=2))
    pt_pool = ctx.enter_context(tc.tile_pool(name="pt", bufs=3))
    c2_pool = ctx.enter_context(tc.tile_pool(name="c2", bufs=3))
    den_pool = ctx.enter_context(tc.tile_pool(name="den", bufs=3))
    scr_pool = ctx.enter_context(tc.tile_pool(name="scr", bufs=6))
    s3_pool = ctx.enter_context(tc.tile_pool(name="s3", bufs=3))
    q2_pool = ctx.enter_context(tc.tile_pool(name="q2", bufs=3))
    sb_small = ctx.enter_context(tc.tile_pool(name="small", bufs=1))

    ps_s_pool = ctx.enter_context(tc.tile_pool(name="pss", bufs=2, space="PSUM"))
    ps_c2q_pool = ctx.enter_context(tc.tile_pool(name="psc", bufs=1, space="PSUM"))
    ps_misc_pool = ctx.enter_context(tc.tile_pool(name="psm", bufs=4, space="PSUM"))

    # ---- constants / weights in SBUF ----
    w_sb = consts.tile([128, 6], F32)          # col = t*2+dh; t: 0=w_c 1=w_q 2=w_m
    nc.gpsimd.dma_start(w_sb[:, :], wsim[:].rearrange("(t dh p) -> p (t dh)", dh=DH, p=128))
    wq_bf = consts.tile([128, DH], BF16)       # w_q as bf16 matmul operand
    nc.vector.tensor_copy(wq_bf[:, :], w_sb[:, 2:4])
    wlab_sb = consts.tile([128, NK, NL], F32)  # chunk k = piece*DH+dh
    nc.gpsimd.dma_start(wlab_sb[:, :, :], wlab[:, :].rearrange("(k p) l -> p k l", p=128))
    b_sb = consts.tile([1, NL], F32)
    nc.gpsimd.dma_start(b_sb[0:1, :], blab[:].rearrange("(o l) -> o l", o=1))
    ones_bf = consts.tile([1, 128], BF16)      # K=1 broadcast lhsT
    nc.vector.memset(ones_bf[0:1, :], 1.0)
    ones128_bf = consts.tile([128, 1], BF16)   # partition-sum lhsT
    nc.vector.memset(ones128_bf[:, :], 1.0)
    ones_f32 = consts.tile([1, 128], F32)      # broadcast lhsT + [1,1] identity
    nc.vector.memset(ones_f32[0:1, :], 1.0)
    id_bf = consts.tile([128, 128], BF16)      # identity for PE transposes
    make_identity(nc, id_bf[:, :])

    # ---- big inputs: cast-load fp32 -> bf16 (SWDGE) ----
    q_dup = bigbuf.tile([128, EX, H2], BF16)        # q on both 64-partition halves
    nc.gpsimd.dma_start(q_dup[0:64, :, :], fq[:, :, :].rearrange("e j d -> j e d"))
    nc.gpsimd.dma_start(q_dup[64:128, :, :], fq[:, :, :].rearrange("e j d -> j e d"))
    c_nat = bigbuf.tile([128, EX, CH, H2], BF16)   # p = i%128
    for g in range(4):                              # 2 examples per DMA
        nc.gpsimd.dma_start(
            c_nat[:, 2 * g:2 * g + 2, :, :],
            fd[2 * g:2 * g + 2, :, :].rearrange("e (ch p) d -> p e ch d", p=128),
        )

    copy_alt = [0]

    def psum_copy(dst_ap, src_ap):
        """PSUM->SBUF copies ride the (less loaded) ACT engine."""
        nc.scalar.copy(dst_ap, src_ap)

    def pe_transpose_group(psum_view, srcs):
        """Transpose each [128|64,128] src into psum_view[:, k, :] via PE."""
        first = None
        for k, src in enumerate(srcs):
            mm = nc.tensor.matmul(
                psum_view[:, k, :], src, id_bf[0:src.shape[0], 0:src.shape[0]],
                is_transpose=True,
                start=(first is None), stop=(k == len(srcs) - 1),
                skip_group_check=True,
            )
            if first is None:
                first = mm
            else:
                add_dep_helper(mm.ins, first.ins, sync=False, reason="bank order")
        return first

    # ---- c^T via PE transposes: [128, EX, DH, C], p = d%128, free = i ----
    c_T = bigbuf.tile([128, EX, DH, C], BF16)
    for e in range(EX):
        for dh in range(DH):
            tp = ps_misc_pool.tile([128, CH, 128], BF16, tag="misc")
            pe_transpose_group(
                tp,
                [c_nat[:, e, chk, dh * 128:(dh + 1) * 128] for chk in range(CH)],
            )
            psum_copy(c_T[:, e, dh, :], tp[:, :, :])

    # ---- q^T via PE transposes: [128, EX, DH, Q], p = d%128, free = j ----
    q_T = bigbuf.tile([128, EX, DH, Q], BF16)
    for pair in range(EX // 2):
        tp = ps_misc_pool.tile([128, 4, Q], BF16, tag="misc")
        srcs = []
        for slot in range(2):
            for dh in range(DH):
                srcs.append(q_dup[0:64, 2 * pair + slot, dh * 128:(dh + 1) * 128])
        pe_transpose_group(tp, srcs)
        psum_copy(
            q_T[:, 2 * pair:2 * pair + 2, :, :],
            tp[:, :, :].rearrange("p (s dh) j -> p s dh j", s=2),
        )

    # ---- persistent per-core accumulators ----
    pm_col = sb_small.tile([128, EX * CH], BF16)    # Pmax^T (b_att numerators), col = e*CH+ch
    final_f = sb_small.tile([128, NK * EX], F32)    # col = (piece*DH+dh)*EX + e
    cmin_f = sb_small.tile([128, DH * EX], F32)     # col = dh*EX + e
    r_sb = sb_small.tile([128, EX], F32)            # 1/sum(pm), broadcast over partitions
    sumb = sb_small.tile([1, EX], F32)
    recipb = sb_small.tile([1, EX], F32)
    out_sb = sb_small.tile([EX, NL], F32)

    fview = final_f[:, :].rearrange("p (pc dh e) -> p pc dh e", pc=4, dh=DH)
    cminv = cmin_f[:, :].rearrange("p (dh e) -> p dh e", dh=DH)

    # ---- piece 0 (max_i c, min_i c) for all examples: batched folds ----
    cT_all = c_T[:, :, :, :]                        # [128, EX, DH, C]
    for op, dst in ((OP.max, fview[:, 0, :, :].rearrange("p dh e -> p e dh")),
                    (OP.min, cminv[:, :, :].rearrange("p dh e -> p e dh"))):
        f1 = scr_pool.tile([128, EX, DH, 256], BF16, tag="bigA")
        nc.vector.tensor_tensor(
            f1[:, :, :, :], cT_all[:, :, :, 0:256], cT_all[:, :, :, 256:512], op=op)
        f2 = scr_pool.tile([128, EX, DH, 128], BF16, tag="bigB")
        nc.vector.tensor_tensor(
            f2[:, :, :, :], f1[:, :, :, 0:128], f1[:, :, :, 128:256], op=op)
        f3 = scr_pool.tile([128, EX, DH, 64], BF16, tag="bigC")
        nc.vector.tensor_tensor(
            f3[:, :, :, :], f2[:, :, :, 0:64], f2[:, :, :, 64:128], op=op)
        nc.vector.tensor_reduce(dst, f3[:, :, :, :], axis=AX.X, op=op)

    dbg_tiles = []
    ps_m_per_ex = {}
    for pair in range(EX // 2):
        p_pair = p_pool.tile([128, CH, 2, Q], BF16)
        pn_pair = pn_pool.tile([128, CH, 2, Q], BF16)

        # ---------- stage A: S + softmax pieces, per slot ----------
        for slot in range(2):
            e = 2 * pair + slot

            # rhs for S-matmul: w_m * q^T + w_c  (folds the c@w_c row term)
            rhs_qm = qm_pool.tile([128, DH, Q], BF16)
            for dh in range(DH):
                nc.gpsimd.tensor_scalar(
                    rhs_qm[:, dh, :], q_T[:, e, dh, :],
                    w_sb[:, 4 + dh:5 + dh], w_sb[:, 0 + dh:1 + dh],
                    op0=OP.mult, op1=OP.add,
                )

            # qw[j] = q @ w_q  (ps_m also hosts q2c regions in stage C)
            ps_m = ps_misc_pool.tile([128, 512], F32, tag="misc")
            ps_m_per_ex[e] = ps_m
            for dh in range(DH):
                nc.tensor.matmul(
                    ps_m[0:1, 0:Q], wq_bf[:, dh:dh + 1], q_T[:, e, dh, :],
                    start=(dh == 0), stop=(dh == DH - 1),
                )
            qwrow = qw_pool.tile([1, Q], BF16)
            nc.vector.tensor_copy(qwrow[0:1, :], ps_m[0:1, 0:Q])

            # S in PSUM: two c^T-chunk matmuls + ones x qw broadcast.
            # start=True clears has_written for the WHOLE bank -> exactly one.
            ps_s = ps_s_pool.tile([128, CH, Q], F32)
            first_mm = None
            for chk in range(CH):
                for dh in range(DH):
                    mm = nc.tensor.matmul(
                        ps_s[:, chk, :],
                        c_T[:, e, dh, chk * 128:(chk + 1) * 128],
                        rhs_qm[:, dh, :],
                        start=(first_mm is None), stop=False,
                        skip_group_check=True,
                    )
                    if first_mm is None:
                        first_mm = mm
                    else:
                        add_dep_helper(mm.ins, first_mm.ins, sync=False,
                                       reason="bank clear order")
            for chk in range(CH):
                mm = nc.tensor.matmul(
                    ps_s[:, chk, :], ones_bf[0:1, :], qwrow[0:1, :],
                    start=False, stop=(chk == CH - 1),
                    skip_group_check=True,
                )
                add_dep_helper(mm.ins, first_mm.ins, sync=False,
                               reason="bank clear order")

            # P = exp(S); row sums; row maxes (-> b_att numerators); P_norm
            nc.scalar.activation(p_pair[:, :, slot, :], ps_s[:, :, :], AF.Exp)
            den = den_pool.tile([128, CH], F32)
            nc.vector.reduce_sum(den[:, :], p_pair[:, :, slot, :], axis=AX.X)
            nc.vector.tensor_reduce(
                pm_col[:, e * CH:(e + 1) * CH], p_pair[:, :, slot, :],
                axis=AX.X, op=OP.max,
            )
            rden = den_pool.tile([128, CH], F32, tag="rden")
            nc.vector.reciprocal(rden[:, :], den[:, :])
            nc.vector.tensor_tensor(
                pn_pair[:, :, slot, :], p_pair[:, :, slot, :],
                rden[:, :].unsqueeze(2).broadcast_to([128, CH, Q]),
                op=OP.mult,
            )

        if DEBUG_DUMP and pair == 0:
            dbgP = sb_small.tile([128, CH * 2 * Q], BF16, tag="dbgP")
            nc.vector.tensor_copy(
                dbgP[:, :].rearrange("p (c s q) -> p c s q", c=CH, s=2),
                p_pair[:, :, :, :])
            dbg_tiles.append(("dbg_P", dbgP, [128, CH * 2 * Q], BF16))

        # ---------- stage B: P^T via PE + b_att normalizers ----------
        pt = pt_pool.tile([128, CH, 128], BF16)     # P_norm^T: [slot*64+j, i(chunk-major)]
        tp = ps_misc_pool.tile([128, CH, 128], BF16, tag="misc")
        pe_transpose_group(tp, [pn_pair[:, chk, :, :] for chk in range(CH)])
        psum_copy(pt[:, :, :], tp[:, :, :])

        e0, e1 = 2 * pair, 2 * pair + 1
        ps_pair = ps_misc_pool.tile([128, 512], F32, tag="misc")
        nc.tensor.matmul(                           # column sums of pm over partitions
            ps_pair[0:1, 0:2 * CH], ones128_bf[:, :],
            pm_col[:, e0 * CH:(e1 + 1) * CH],
            start=True, stop=True,
        )
        nc.vector.reduce_sum(
            sumb[0:1, e0:e1 + 1],
            ps_pair[0:1, 0:2 * CH].rearrange("o (e c) -> o e c", c=CH),
            axis=AX.X,
        )
        nc.vector.reciprocal(recipb[0:1, e0:e1 + 1], sumb[0:1, e0:e1 + 1])
        nc.tensor.matmul(                           # broadcast 1/sum to all partitions
            ps_pair[:, 8:10], ones_f32[0:1, :], recipb[0:1, e0:e1 + 1],
            start=True, stop=True,
        )
        nc.vector.tensor_copy(r_sb[:, e0:e1 + 1], ps_pair[:, 8:10])

        # ---------- stage C: c2q + max-pools, per slot ----------
        for slot in range(2):
            e = 2 * pair + slot

            # c2q^T = q^T @ P_norm^T   [128(d), 512(i)] per d-chunk
            ps_c2q = ps_c2q_pool.tile([128, DH, C], F32)
            for dh in range(DH):
                nc.tensor.matmul(
                    ps_c2q[:, dh, :],
                    q_dup[slot * 64:slot * 64 + 64, e, dh * 128:(dh + 1) * 128],
                    pt[slot * 64:slot * 64 + 64, :, :],
                    start=True, stop=True,
                    tile_position=(slot * 64, 0),
                )
            c2q_sb = c2_pool.tile([128, DH, C], BF16)
            nc.scalar.copy(c2q_sb[:, :, :], ps_c2q[:, :, :])

            # piece 1 (max of c2q): 2x-mode TT folds + short 1x reduce.
            scrA = scr_pool.tile([128, DH, 256], BF16, tag="scrA")
            scrB = scr_pool.tile([128, DH, 128], BF16, tag="scrB")
            for (src, op, dst) in (
                (c2q_sb[:, :, :], OP.max, fview[:, 1, :, e]),
            ):
                nc.vector.tensor_tensor(
                    scrA[:, :, :], src[:, :, 0:256], src[:, :, 256:512], op=op)
                nc.vector.tensor_tensor(
                    scrB[:, :, :], scrA[:, :, 0:128], scrA[:, :, 128:256], op=op)
                nc.vector.tensor_reduce(dst, scrB[:, :, :], axis=AX.X, op=op)

            # piece 2: max_i (c * c2q)
            prod = scr_pool.tile([128, DH, C], BF16, tag="prod")
            nc.vector.tensor_tensor(
                prod[:, :, :], c_T[:, e, :, :], c2q_sb[:, :, :], op=OP.mult)
            nc.vector.tensor_tensor(
                scrA[:, :, :], prod[:, :, 0:256], prod[:, :, 256:512], op=OP.max)
            nc.vector.tensor_tensor(
                scrB[:, :, :], scrA[:, :, 0:128], scrA[:, :, 128:256], op=OP.max)
            nc.vector.tensor_reduce(fview[:, 2, :, e], scrB[:, :, :], axis=AX.X, op=OP.max)

            # q2c = b_att @ c  (unnormalized weights pm, scale by 1/sum after)
            ps_m2 = ps_m_per_ex[e]
            for chk in range(CH):
                nc.tensor.matmul(
                    ps_m2[0:1, 64:64 + H2],
                    pm_col[:, e * CH + chk:e * CH + chk + 1],
                    c_nat[:, e, chk, :],
                    start=(chk == 0), stop=(chk == CH - 1),
                )
            q2c_sc = q2_pool.tile([1, H2], F32)
            nc.scalar.mul(q2c_sc[0:1, :], ps_m2[0:1, 64:64 + H2], r_sb[0:1, e:e + 1])
            # transpose q2c to [128, DH] via PE (identity = [[1.0]])
            for dh in range(DH):
                nc.tensor.matmul(
                    ps_m2[:, 320 + dh:321 + dh],
                    q2c_sc[0:1, dh * 128:(dh + 1) * 128],
                    ones_f32[0:1, 0:1],
                    is_transpose=True,
                    start=(dh == 0), stop=(dh == DH - 1),
                    skip_group_check=True,
                )
            # piece 3: max_i (c * q2c) = max(q2c*cmax, q2c*cmin)
            s3 = s3_pool.tile([128, 4], F32)
            nc.vector.tensor_tensor(
                s3[:, 0:2], ps_m2[:, 320:322], fview[:, 0, :, e], op=OP.mult)
            nc.vector.tensor_tensor(
                s3[:, 2:4], ps_m2[:, 320:322], cminv[:, :, e], op=OP.mult)
            nc.vector.tensor_tensor(
                fview[:, 3, :, e], s3[:, 0:2], s3[:, 2:4], op=OP.max)

    # ---------- final: out = max-pooled features @ w_label + b ----------
    ps_out = ps_misc_pool.tile([128, 512], F32, tag="misc")
    for k in range(NK):
        nc.tensor.matmul(
            ps_out[0:EX, 0:NL], final_f[:, k * EX:(k + 1) * EX], wlab_sb[:, k, :],
            start=(k == 0), stop=False, skip_group_check=True,
        )
    nc.tensor.matmul(
        ps_out[0:EX, 0:NL], ones_f32[0:1, 0:EX], b_sb[0:1, :],
        start=False, stop=True, skip_group_check=True,
    )
    nc.vector.tensor_copy(out_sb[:, :], ps_out[0:EX, 0:NL])
    nc.sync.dma_start(out[:, :], out_sb[:, :])

    if DEBUG_DUMP:
        for name, tl, shp, dt in [
            ("dbg_final", final_f, [128, NK * EX], F32),
            ("dbg_cmin", cmin_f, [128, DH * EX], F32),
            ("dbg_pm", pm_col, [128, EX * CH], BF16),
            ("dbg_r", r_sb, [128, EX], F32),
            ("dbg_sumb", sumb, [1, EX], F32),
        ] + dbg_tiles:
            t = nc.dram_tensor(name, shp, dt, kind="ExternalOutput")
            nc.sync.dma_start(t[:, :], tl[:, :])


def build_nc():
    nc = bacc.Bacc("TRN2", target_bir_lowering=False, debug=False)
    fd = nc.dram_tensor("fd", [EX, C, H2], F32, kind="ExternalInput")
    fq = nc.dram_tensor("fq", [EX, Q, H2], F32, kind="ExternalInput")
    wsim = nc.dram_tensor("wsim", [3 * H2], F32, kind="ExternalInput")
    wlab = nc.dram_tensor("wlab", [4 * H2, NL], F32, kind="ExternalInput")
    blab = nc.dram_tensor("blab", [NL], F32, kind="ExternalInput")
    out = nc.dram_tensor("out", [EX, NL], F32, kind="ExternalOutput")

    from contextlib import ExitStack
    with tile.TileContext(nc) as tc:
        with ExitStack() as ctx:
            _body(tc, ctx, fd[:, :, :], fq[:, :, :], wsim[:], wlab[:, :], blab[:], out[:, :])
    nc.compile()
    return nc


_NC_CACHE = None


def run(inputs, trace=False):
    global _NC_CACHE
    if _NC_CACHE is None:
        _NC_CACHE = build_nc()
    nc = _NC_CACHE

    fd = np.ascontiguousarray(np.asarray(inputs["feature_document"], dtype=np.float32))
    fq = np.ascontiguousarray(np.asarray(inputs["feature_query"], dtype=np.float32))
    wsim = np.ascontiguousarray(np.asarray(inputs["w_sim"], dtype=np.float32))
    wlab = np.ascontiguousarray(np.asarray(inputs["w_label"], dtype=np.float32))
    blab = np.ascontiguousarray(np.asarray(inputs["b_label"], dtype=np.float32))

    in_maps = []
    for core in range(N_CORES):
        sl = slice(core * EX, (core + 1) * EX)
        in_maps.append({
            "fd": fd[sl], "fq": fq[sl],
            "wsim": wsim, "wlab": wlab, "blab": blab,
        })
    res = run_bass_kernel_spmd(nc, in_maps, list(range(N_CORES)), trace=trace)
    outs = np.concatenate([np.asarray(res.results[i]["out"]) for i in range(N_CORES)], axis=0)
    return outs.astype(np.float32), res


def kernel(**inputs):
    outs, _ = run(inputs, trace=False)
    return outs


# revision 37
# speedup vs baseline: 1.0520x; 1.0520x over previous
"""BiDAF attention-flow kernel for Trainium2 (8 NeuronCores, data-parallel).

Self-contained: hardcodes shapes B,C,Q,H2 = 64,512,64,256; n_labels=2.
kernel(**inputs) takes full unsharded inputs, shards batch over 8 cores,
runs one SPMD Bass/Tile kernel, gathers [8,2] per core -> [64,2].

Per-core math (8 examples, bf16 compute, fp32 accumulation):
  S = c @ diag(w_m) @ q^T + (c@w_c)[:,None] + (q@w_q)[None,:]
    - the c@w_c term folds into the matmul rhs (rhs = w_m*q^T + w_c),
    - the q@w_q term rides in via a K=1 all-ones broadcast matmul.
  P = exp(S) unstabilized (|S| is O(1) for this distribution), so
  row-softmax needs only row-sums, and b_att = softmax(max_j S) is just
  Pmax/sum(Pmax) with Pmax = max_j P  (exp is monotone).
  All transposes (c^T, q^T, P_norm^T, q2c^T) go through the PE transpose
  path (DMA xbar transposes measured ~1.2us each, serial on the sync
  queue).  Max-pools over context run in the d-major layout as 2x-mode
  tensor_tensor max folds plus a short 1x reduce; the c*q2c piece uses
  max(q2c*cmax, q2c*cmin) so it needs no extra pass over the data.
"""

import os
import sys

for _p in ("/opt/trn_rl_repo", "/opt/pypackages"):
    if os.path.isdir(_p) and _p not in sys.path:
        sys.path.insert(0, _p)

import numpy as np

import concourse.bass as bass
import concourse.bacc as bacc
import concourse.tile as tile
import concourse.mybir as mybir
from concourse.bass_utils import run_bass_kernel_spmd
from concourse.masks import make_identity
from concourse.tile_rust import add_dep_helper

F32 = mybir.dt.float32
BF16 = mybir.dt.bfloat16
AX = mybir.AxisListType
OP = mybir.AluOpType
AF = mybir.ActivationFunctionType

DEBUG_DUMP = False

N_CORES = 8
B, C, Q, H2 = 64, 512, 64, 256
NL = 2
EX = B // N_CORES          # examples per core = 8
CH = C // 128              # context chunks of 128 = 4
DH = H2 // 128             # feature chunks of 128 = 2
NK = 4 * DH                # final feature chunks (4 pieces x DH) = 8


def _body(tc, ctx, fd, fq, wsim, wlab, blab, out):
    nc = tc.nc

    consts = ctx.enter_context(tc.tile_pool(name="consts", bufs=1))
    bigbuf = ctx.enter_context(tc.tile_pool(name="bigbuf", bufs=1))
    qm_pool = ctx.enter_context(tc.tile_pool(name="qm", bufs=3))
    qw_pool = ctx.enter_context(tc.tile_pool(name="qw", bufs=3))
    p_pool = ctx.enter_context(tc.tile_pool(name="p", bufs=2))
    pn_pool = ctx.enter_context(tc.tile_pool(name="pn", bufs=2))
    pt_pool = ctx.enter_context(tc.tile_pool(name="pt", bufs=3))
    c2_pool = ctx.enter_context(tc.tile_pool(name="c2", bufs=3))
    den_pool = ctx.enter_context(tc.tile_pool(name="den", bufs=3))
    scr_pool = ctx.enter_context(tc.tile_pool(name="scr", bufs=6))
    s3_pool = ctx.enter_context(tc.tile_pool(name="s3", bufs=3))
    q2_pool = ctx.enter_context(tc.tile_pool(name="q2", bufs=3))
    sb_small = ctx.enter_context(tc.tile_pool(name="small", bufs=1))

    ps_s_pool = ctx.enter_context(tc.tile_pool(name="pss", bufs=2, space="PSUM"))
    ps_c2q_pool = ctx.enter_context(tc.tile_pool(name="psc", bufs=1, space="PSUM"))
    ps_misc_pool = ctx.enter_context(tc.tile_pool(name="psm", bufs=4, space="PSUM"))

    # ---- constants / weights in SBUF ----
    w_sb = consts.tile([128, 6], F32)          # col = t*2+dh; t: 0=w_c 1=w_q 2=w_m
    nc.gpsimd.dma_start(w_sb[:, :], wsim[:].rearrange("(t dh p) -> p (t dh)", dh=DH, p=128))
    wq_bf = consts.tile([128, DH], BF16)       # w_q as bf16 matmul operand
    nc.vector.tensor_copy(wq_bf[:, :], w_sb[:, 2:4])
    wlab_sb = consts.tile([128, NK, NL], F32)  # chunk k = piece*DH+dh
    nc.gpsimd.dma_start(wlab_sb[:, :, :], wlab[:, :].rearrange("(k p) l -> p k l", p=128))
    b_sb = consts.tile([1, NL], F32)
    nc.gpsimd.dma_start(b_sb[0:1, :], blab[:].rearrange("(o l) -> o l", o=1))
    ones_bf = consts.tile([1, 128], BF16)      # K=1 broadcast lhsT
    nc.vector.memset(ones_bf[0:1, :], 1.0)
    ones128_bf = consts.tile([128, 1], BF16)   # partition-sum lhsT
    nc.vector.memset(ones128_bf[:, :], 1.0)
    ones_f32 = consts.tile([1, 128], F32)      # broadcast lhsT + [1,1] identity
    nc.vector.memset(ones_f32[0:1, :], 1.0)
    id_bf = consts.tile([128, 128], BF16)      # identity for PE transposes
    make_identity(nc, id_bf[:, :])

    # ---- big inputs: cast-load fp32 -> bf16 (SWDGE) ----
    q_dup = bigbuf.tile([128, EX, H2], BF16)        # q on both 64-partition halves
    nc.gpsimd.dma_start(q_dup[0:64, :, :], fq[:, :, :].rearrange("e j d -> j e d"))
    nc.gpsimd.dma_start(q_dup[64:128, :, :], fq[:, :, :].rearrange("e j d -> j e d"))
    c_nat = bigbuf.tile([128, EX, CH, H2], BF16)   # p = i%128
    for g in range(4):                              # 2 examples per DMA
        nc.gpsimd.dma_start(
            c_nat[:, 2 * g:2 * g + 2, :, :],
            fd[2 * g:2 * g + 2, :, :].rearrange("e (ch p) d -> p e ch d", p=128),
        )

    copy_alt = [0]

    def psum_copy(dst_ap, src_ap):
        """PSUM->SBUF copies ride the (less loaded) ACT engine."""
        nc.scalar.copy(dst_ap, src_ap)

    def pe_transpose_group(psum_view, srcs):
        """Transpose each [128|64,128] src into psum_view[:, k, :] via PE."""
        first = None
        for k, src in enumerate(srcs):
            mm = nc.tensor.matmul(
                psum_view[:, k, :], src, id_bf[0:src.shape[0], 0:src.shape[0]],
                is_transpose=True,
                start=(first is None), stop=(k == len(srcs) - 1),
                skip_group_check=True,
            )
            if first is None:
                first = mm
            else:
                add_dep_helper(mm.ins, first.ins, sync=False, reason="bank order")
        return first

    # ---- c^T via PE transposes: [128, EX, DH, C], p = d%128, free = i ----
    c_T = bigbuf.tile([128, EX, DH, C], BF16)
    for e in range(EX):
        for dh in range(DH):
            tp = ps_misc_pool.tile([128, CH, 128], BF16, tag="misc")
            pe_transpose_group(
                tp,
                [c_nat[:, e, chk, dh * 128:(dh + 1) * 128] for chk in range(CH)],
            )
            psum_copy(c_T[:, e, dh, :], tp[:, :, :])

    # ---- q^T via PE transposes: [128, EX, DH, Q], p = d%128, free = j ----
    q_T = bigbuf.tile([128, EX, DH, Q], BF16)
    for pair in range(EX // 2):
        tp = ps_misc_pool.tile([128, 4, Q], BF16, tag="misc")
        srcs = []
        for slot in range(2):
            for dh in range(DH):
                srcs.append(q_dup[0:64, 2 * pair + slot, dh * 128:(dh + 1) * 128])
        pe_transpose_group(tp, srcs)
        psum_copy(
            q_T[:, 2 * pair:2 * pair + 2, :, :],
            tp[:, :, :].rearrange("p (s dh) j -> p s dh j", s=2),
        )

    # ---- persistent per-core accumulators ----
    pm_col = sb_small.tile([128, EX * CH], BF16)    # Pmax^T (b_att numerators), col = e*CH+ch
    final_f = sb_small.tile([128, NK * EX], F32)    # col = (piece*DH+dh)*EX + e
    cmin_f = sb_small.tile([128, DH * EX], F32)     # col = dh*EX + e
    r_sb = sb_small.tile([128, EX], F32)            # 1/sum(pm), broadcast over partitions
    sumb = sb_small.tile([1, EX], F32)
    recipb = sb_small.tile([1, EX], F32)
    out_sb = sb_small.tile([EX, NL], F32)
    q2cT_sb = sb_small.tile([128, EX, DH], F32)

    fview = final_f[:, :].rearrange("p (pc dh e) -> p pc dh e", pc=4, dh=DH)
    cminv = cmin_f[:, :].rearrange("p (dh e) -> p dh e", dh=DH)

    dbg_tiles = []
    ps_m_per_ex = {}
    for pair in range(EX // 2):
        p_pair = p_pool.tile([128, CH, 2, Q], BF16)
        pn_pair = pn_pool.tile([128, CH, 2, Q], BF16)

        # ---------- stage A: S + softmax pieces, per slot ----------
        for slot in range(2):
            e = 2 * pair + slot

            # rhs for S-matmul: w_m * q^T + w_c  (folds the c@w_c row term)
            rhs_qm = qm_pool.tile([128, DH, Q], BF16)
            for dh in range(DH):
                nc.gpsimd.tensor_scalar(
                    rhs_qm[:, dh, :], q_T[:, e, dh, :],
                    w_sb[:, 4 + dh:5 + dh], w_sb[:, 0 + dh:1 + dh],
                    op0=OP.mult, op1=OP.add,
                )

            # qw[j] = q @ w_q  (ps_m also hosts q2c regions in stage C)
            ps_m = ps_misc_pool.tile([128, 512], F32, tag="misc")
            ps_m_per_ex[e] = ps_m
            for dh in range(DH):
                nc.tensor.matmul(
                    ps_m[0:1, 0:Q], wq_bf[:, dh:dh + 1], q_T[:, e, dh, :],
                    start=(dh == 0), stop=(dh == DH - 1),
                )
            qwrow = qw_pool.tile([1, Q], BF16)
            nc.vector.tensor_copy(qwrow[0:1, :], ps_m[0:1, 0:Q])

            # S in PSUM: two c^T-chunk matmuls + ones x qw broadcast.
            # start=True clears has_written for the WHOLE bank -> exactly one.
            ps_s = ps_s_pool.tile([128, CH, Q], F32)
            first_mm = None
            for chk in range(CH):
                for dh in range(DH):
                    mm = nc.tensor.matmul(
                        ps_s[:, chk, :],
                        c_T[:, e, dh, chk * 128:(chk + 1) * 128],
                        rhs_qm[:, dh, :],
                        start=(first_mm is None), stop=False,
                        skip_group_check=True,
                    )
                    if first_mm is None:
                        first_mm = mm
                    else:
                        add_dep_helper(mm.ins, first_mm.ins, sync=False,
                                       reason="bank clear order")
            for chk in range(CH):
                mm = nc.tensor.matmul(
                    ps_s[:, chk, :], ones_bf[0:1, :], qwrow[0:1, :],
                    start=False, stop=(chk == CH - 1),
                    skip_group_check=True,
                )
                add_dep_helper(mm.ins, first_mm.ins, sync=False,
                               reason="bank clear order")

            # P = exp(S); row sums; row maxes (-> b_att numerators); P_norm
            nc.scalar.activation(p_pair[:, :, slot, :], ps_s[:, :, :], AF.Exp)
            den = den_pool.tile([128, CH], F32)
            nc.vector.reduce_sum(den[:, :], p_pair[:, :, slot, :], axis=AX.X)
            nc.vector.tensor_reduce(
                pm_col[:, e * CH:(e + 1) * CH], p_pair[:, :, slot, :],
                axis=AX.X, op=OP.max,
            )
            rden = den_pool.tile([128, CH], F32, tag="rden")
            nc.vector.reciprocal(rden[:, :], den[:, :])
            nc.vector.tensor_tensor(
                pn_pair[:, :, slot, :], p_pair[:, :, slot, :],
                rden[:, :].unsqueeze(2).broadcast_to([128, CH, Q]),
                op=OP.mult,
            )

        if DEBUG_DUMP and pair == 0:
            dbgP = sb_small.tile([128, CH * 2 * Q], BF16, tag="dbgP")
            nc.vector.tensor_copy(
                dbgP[:, :].rearrange("p (c s q) -> p c s q", c=CH, s=2),
                p_pair[:, :, :, :])
            dbg_tiles.append(("dbg_P", dbgP, [128, CH * 2 * Q], BF16))

        # ---------- stage B: P^T via PE + b_att normalizers ----------
        pt = pt_pool.tile([128, CH, 128], BF16)     # P_norm^T: [slot*64+j, i(chunk-major)]
        tp = ps_misc_pool.tile([128, CH, 128], BF16, tag="misc")
        pe_transpose_group(tp, [pn_pair[:, chk, :, :] for chk in range(CH)])
        psum_copy(pt[:, :, :], tp[:, :, :])

        e0, e1 = 2 * pair, 2 * pair + 1
        ps_pair = ps_misc_pool.tile([128, 512], F32, tag="misc")
        nc.tensor.matmul(                           # column sums of pm over partitions
            ps_pair[0:1, 0:2 * CH], ones128_bf[:, :],
            pm_col[:, e0 * CH:(e1 + 1) * CH],
            start=True, stop=True,
        )
        nc.vector.reduce_sum(
            sumb[0:1, e0:e1 + 1],
            ps_pair[0:1, 0:2 * CH].rearrange("o (e c) -> o e c", c=CH),
            axis=AX.X,
        )
        nc.vector.reciprocal(recipb[0:1, e0:e1 + 1], sumb[0:1, e0:e1 + 1])
        nc.tensor.matmul(                           # broadcast 1/sum to all partitions
            ps_pair[:, 8:10], ones_f32[0:1, :], recipb[0:1, e0:e1 + 1],
            start=True, stop=True,
        )
        nc.vector.tensor_copy(r_sb[:, e0:e1 + 1], ps_pair[:, 8:10])

        # ---------- stage C: c2q + max-pools, per slot ----------
        for slot in range(2):
            e = 2 * pair + slot

            # c2q^T = q^T @ P_norm^T   [128(d), 512(i)] per d-chunk
            ps_c2q = ps_c2q_pool.tile([128, DH, C], F32)
            for dh in range(DH):
                nc.tensor.matmul(
                    ps_c2q[:, dh, :],
                    q_dup[slot * 64:slot * 64 + 64, e, dh * 128:(dh + 1) * 128],
                    pt[slot * 64:slot * 64 + 64, :, :],
                    start=True, stop=True,
                    tile_position=(slot * 64, 0),
                )
            c2q_sb = c2_pool.tile([128, DH, C], BF16)
            nc.scalar.copy(c2q_sb[:, :, :], ps_c2q[:, :, :])

            # piece 1 (max of c2q): 2x-mode TT folds + short 1x reduce.
            scrA = scr_pool.tile([128, DH, 256], BF16, tag="scrA")
            scrB = scr_pool.tile([128, DH, 128], BF16, tag="scrB")
            for (src, op, dst) in (
                (c2q_sb[:, :, :], OP.max, fview[:, 1, :, e]),
            ):
                nc.vector.tensor_tensor(
                    scrA[:, :, :], src[:, :, 0:256], src[:, :, 256:512], op=op)
                nc.vector.tensor_tensor(
                    scrB[:, :, :], scrA[:, :, 0:128], scrA[:, :, 128:256], op=op)
                nc.vector.tensor_reduce(dst, scrB[:, :, :], axis=AX.X, op=op)

            # piece 2: max_i (c * c2q)
            prod = scr_pool.tile([128, DH, C], BF16, tag="prod")
            nc.vector.tensor_tensor(
                prod[:, :, :], c_T[:, e, :, :], c2q_sb[:, :, :], op=OP.mult)
            nc.vector.tensor_tensor(
                scrA[:, :, :], prod[:, :, 0:256], prod[:, :, 256:512], op=OP.max)
            nc.vector.tensor_tensor(
                scrB[:, :, :], scrA[:, :, 0:128], scrA[:, :, 128:256], op=OP.max)
            nc.vector.tensor_reduce(fview[:, 2, :, e], scrB[:, :, :], axis=AX.X, op=OP.max)

            # q2c = b_att @ c  (unnormalized weights pm, scale by 1/sum after)
            ps_m2 = ps_m_per_ex[e]
            for chk in range(CH):
                nc.tensor.matmul(
                    ps_m2[0:1, 64:64 + H2],
                    pm_col[:, e * CH + chk:e * CH + chk + 1],
                    c_nat[:, e, chk, :],
                    start=(chk == 0), stop=(chk == CH - 1),
                )
            q2c_sc = q2_pool.tile([1, H2], F32)
            nc.scalar.mul(q2c_sc[0:1, :], ps_m2[0:1, 64:64 + H2], r_sb[0:1, e:e + 1])
            # transpose q2c to [128, DH] via PE (identity = [[1.0]])
            for dh in range(DH):
                nc.tensor.matmul(
                    ps_m2[:, 320 + dh:321 + dh],
                    q2c_sc[0:1, dh * 128:(dh + 1) * 128],
                    ones_f32[0:1, 0:1],
                    is_transpose=True,
                    start=(dh == 0), stop=(dh == DH - 1),
                    skip_group_check=True,
                )
            # stash q2c^T; piece 3 is computed batched at the end
            nc.scalar.copy(q2cT_sb[:, e, :], ps_m2[:, 320:322])

    # ---- piece 0 (max_i c, min_i c) for all examples: batched folds ----
    cT_all = c_T[:, :, :, :]                        # [128, EX, DH, C]
    for op, dst in ((OP.max, fview[:, 0, :, :].rearrange("p dh e -> p e dh")),
                    (OP.min, cminv[:, :, :].rearrange("p dh e -> p e dh"))):
        f1 = scr_pool.tile([128, EX, DH, 256], BF16, tag="bigA")
        nc.vector.tensor_tensor(
            f1[:, :, :, :], cT_all[:, :, :, 0:256], cT_all[:, :, :, 256:512], op=op)
        f2 = scr_pool.tile([128, EX, DH, 128], BF16, tag="bigB")
        nc.vector.tensor_tensor(
            f2[:, :, :, :], f1[:, :, :, 0:128], f1[:, :, :, 128:256], op=op)
        f3 = scr_pool.tile([128, EX, DH, 64], BF16, tag="bigC")
        nc.vector.tensor_tensor(
            f3[:, :, :, :], f2[:, :, :, 0:64], f2[:, :, :, 64:128], op=op)
        nc.vector.tensor_reduce(dst, f3[:, :, :, :], axis=AX.X, op=op)

    # ---- piece 3 batched: max(q2c*cmax, q2c*cmin) over all examples ----
    s3a = s3_pool.tile([128, EX, DH], F32, tag="s3a")
    s3b = s3_pool.tile([128, EX, DH], F32, tag="s3b")
    nc.vector.tensor_tensor(
        s3a[:, :, :], q2cT_sb[:, :, :],
        fview[:, 0, :, :].rearrange("p dh e -> p e dh"), op=OP.mult)
    nc.vector.tensor_tensor(
        s3b[:, :, :], q2cT_sb[:, :, :],
        cminv[:, :, :].rearrange("p dh e -> p e dh"), op=OP.mult)
    nc.vector.tensor_tensor(
        fview[:, 3, :, :].rearrange("p dh e -> p e dh"), s3a[:, :, :], s3b[:, :, :],
        op=OP.max)

    # ---------- final: out = max-pooled features @ w_label + b ----------
    ps_out = ps_misc_pool.tile([128, 512], F32, tag="misc")
    for k in range(NK):
        nc.tensor.matmul(
            ps_out[0:EX, 0:NL], final_f[:, k * EX:(k + 1) * EX], wlab_sb[:, k, :],
            start=(k == 0), stop=False, skip_group_check=True,
        )
    nc.tensor.matmul(
        ps_out[0:EX, 0:NL], ones_f32[0:1, 0:EX], b_sb[0:1, :],
        start=False, stop=True, skip_group_check=True,
    )
    nc.vector.tensor_copy(out_sb[:, :], ps_out[0:EX, 0:NL])
    nc.sync.dma_start(out[:, :], out_sb[:, :])

    if DEBUG_DUMP:
        for name, tl, shp, dt in [
            ("dbg_final", final_f, [128, NK * EX], F32),
            ("dbg_cmin", cmin_f, [128, DH * EX], F32),
            ("dbg_pm", pm_col, [128, EX * CH], BF16),
            ("dbg_r", r_sb, [128, EX], F32),
            ("dbg_sumb", sumb, [1, EX], F32),
        ] + dbg_tiles:
            t = nc.dram_tensor(name, shp, dt, kind="ExternalOutput")
            nc.sync.dma_start(t[:, :], tl[:, :])


def build_nc():
    nc = bacc.Bacc("TRN2", target_bir_lowering=False, debug=False)
    fd = nc.dram_tensor("fd", [EX, C, H2], F32, kind="ExternalInput")
    fq = nc.dram_tensor("fq", [EX, Q, H2], F32, kind="ExternalInput")
    wsim = nc.dram_tensor("wsim", [3 * H2], F32, kind="ExternalInput")
    wlab = nc.dram_tensor("wlab", [4 * H2, NL], F32, kind="ExternalInput")
    blab = nc.dram_tensor("blab", [NL], F32, kind="ExternalInput")
    out = nc.dram_tensor("out", [EX, NL], F32, kind="ExternalOutput")

    from contextlib import ExitStack
    with tile.TileContext(nc) as tc:
        with ExitStack() as ctx:
            _body(tc, ctx, fd[:, :, :], fq[:, :, :], wsim[:], wlab[:, :], blab[:], out[:, :])
    nc.compile()
    return nc


_NC_CACHE = None


def run(inputs, trace=False):
    global _NC_CACHE
    if _NC_CACHE is None:
        _NC_CACHE = build_nc()
    nc = _NC_CACHE

    fd = np.ascontiguousarray(np.asarray(inputs["feature_document"], dtype=np.float32))
    fq = np.ascontiguousarray(np.asarray(inputs["feature_query"], dtype=np.float32))
    wsim = np.ascontiguousarray(np.asarray(inputs["w_sim"], dtype=np.float32))
    wlab = np.ascontiguousarray(np.asarray(inputs["w_label"], dtype=np.float32))
    blab = np.ascontiguousarray(np.asarray(inputs["b_label"], dtype=np.float32))

    in_maps = []
    for core in range(N_CORES):
        sl = slice(core * EX, (core + 1) * EX)
        in_maps.append({
            "fd": fd[sl], "fq": fq[sl],
            "wsim": wsim, "wlab": wlab, "blab": blab,
        })
    res = run_bass_kernel_spmd(nc, in_maps, list(range(N_CORES)), trace=trace)
    outs = np.concatenate([np.asarray(res.results[i]["out"]) for i in range(N_CORES)], axis=0)
    return outs.astype(np.float32), res


def kernel(**inputs):
    outs, _ = run(inputs, trace=False)
    return outs


# revision 38
# speedup vs baseline: 1.1131x; 1.0581x over previous
"""BiDAF attention-flow kernel for Trainium2 (8 NeuronCores, data-parallel).

Self-contained: hardcodes shapes B,C,Q,H2 = 64,512,64,256; n_labels=2.
kernel(**inputs) takes full unsharded inputs, shards batch over 8 cores,
runs one SPMD Bass/Tile kernel, gathers [8,2] per core -> [64,2].

Per-core math (8 examples, bf16 compute, fp32 accumulation):
  S = c @ diag(w_m) @ q^T + (c@w_c)[:,None] + (q@w_q)[None,:]
    - the c@w_c term folds into the matmul rhs (rhs = w_m*q^T + w_c),
    - the q@w_q term rides in via a K=1 all-ones broadcast matmul.
  P = exp(S) unstabilized (|S| is O(1) for this distribution), so
  row-softmax needs only row-sums, and b_att = softmax(max_j S) is just
  Pmax/sum(Pmax) with Pmax = max_j P  (exp is monotone).
  All transposes (c^T, q^T, P_norm^T, q2c^T) go through the PE transpose
  path (DMA xbar transposes measured ~1.2us each, serial on the sync
  queue).  Max-pools over context run in the d-major layout as 2x-mode
  tensor_tensor max folds plus a short 1x reduce; the c*q2c piece uses
  max(q2c*cmax, q2c*cmin) so it needs no extra pass over the data.
"""

import os
import sys

for _p in ("/opt/trn_rl_repo", "/opt/pypackages"):
    if os.path.isdir(_p) and _p not in sys.path:
        sys.path.insert(0, _p)

import numpy as np

import concourse.bass as bass
import concourse.bacc as bacc
import concourse.tile as tile
import concourse.mybir as mybir
from concourse.bass_utils import run_bass_kernel_spmd
from concourse.masks import make_identity
from concourse.tile_rust import add_dep_helper

F32 = mybir.dt.float32
BF16 = mybir.dt.bfloat16
AX = mybir.AxisListType
OP = mybir.AluOpType
AF = mybir.ActivationFunctionType

DEBUG_DUMP = False

N_CORES = 8
B, C, Q, H2 = 64, 512, 64, 256
NL = 2
EX = B // N_CORES          # examples per core = 8
CH = C // 128              # context chunks of 128 = 4
DH = H2 // 128             # feature chunks of 128 = 2
NK = 4 * DH                # final feature chunks (4 pieces x DH) = 8


def _body(tc, ctx, fd, fq, wsim, wlab, blab, out):
    nc = tc.nc

    consts = ctx.enter_context(tc.tile_pool(name="consts", bufs=1))
    bigbuf = ctx.enter_context(tc.tile_pool(name="bigbuf", bufs=1))
    qm_pool = ctx.enter_context(tc.tile_pool(name="qm", bufs=3))
    qw_pool = ctx.enter_context(tc.tile_pool(name="qw", bufs=3))
    p_pool = ctx.enter_context(tc.tile_pool(name="p", bufs=2))
    pn_pool = ctx.enter_context(tc.tile_pool(name="pn", bufs=2))
    pt_pool = ctx.enter_context(tc.tile_pool(name="pt", bufs=3))
    c2_pool = ctx.enter_context(tc.tile_pool(name="c2", bufs=3))
    den_pool = ctx.enter_context(tc.tile_pool(name="den", bufs=3))
    scr_pool = ctx.enter_context(tc.tile_pool(name="scr", bufs=6))
    s3_pool = ctx.enter_context(tc.tile_pool(name="s3", bufs=3))
    q2_pool = ctx.enter_context(tc.tile_pool(name="q2", bufs=3))
    sb_small = ctx.enter_context(tc.tile_pool(name="small", bufs=1))

    ps_s_pool = ctx.enter_context(tc.tile_pool(name="pss", bufs=2, space="PSUM"))
    ps_c2q_pool = ctx.enter_context(tc.tile_pool(name="psc", bufs=1, space="PSUM"))
    ps_misc_pool = ctx.enter_context(tc.tile_pool(name="psm", bufs=4, space="PSUM"))

    # ---- constants / weights in SBUF ----
    w_sb = consts.tile([128, 6], F32)          # col = t*2+dh; t: 0=w_c 1=w_q 2=w_m
    nc.gpsimd.dma_start(w_sb[:, :], wsim[:].rearrange("(t dh p) -> p (t dh)", dh=DH, p=128))
    wq_bf = consts.tile([128, DH], BF16)       # w_q as bf16 matmul operand
    nc.vector.tensor_copy(wq_bf[:, :], w_sb[:, 2:4])
    wlab_sb = consts.tile([128, NK, NL], F32)  # chunk k = piece*DH+dh
    nc.gpsimd.dma_start(wlab_sb[:, :, :], wlab[:, :].rearrange("(k p) l -> p k l", p=128))
    b_sb = consts.tile([1, NL], F32)
    nc.gpsimd.dma_start(b_sb[0:1, :], blab[:].rearrange("(o l) -> o l", o=1))
    ones_bf = consts.tile([1, 128], BF16)      # K=1 broadcast lhsT
    nc.vector.memset(ones_bf[0:1, :], 1.0)
    ones128_bf = consts.tile([128, 1], BF16)   # partition-sum lhsT
    nc.vector.memset(ones128_bf[:, :], 1.0)
    ones_f32 = consts.tile([1, 128], F32)      # broadcast lhsT + [1,1] identity
    nc.vector.memset(ones_f32[0:1, :], 1.0)
    id_bf = consts.tile([128, 128], BF16)      # identity for PE transposes
    make_identity(nc, id_bf[:, :])

    # ---- big inputs: cast-load fp32 -> bf16 (SWDGE) ----
    q_dup = bigbuf.tile([128, EX, H2], BF16)        # q on both 64-partition halves
    nc.gpsimd.dma_start(q_dup[0:64, :, :], fq[:, :, :].rearrange("e j d -> j e d"))
    nc.gpsimd.dma_start(q_dup[64:128, :, :], fq[:, :, :].rearrange("e j d -> j e d"))
    c_nat = bigbuf.tile([128, EX, CH, H2], BF16)   # p = i%128
    for g in range(4):                              # 2 examples per DMA
        nc.gpsimd.dma_start(
            c_nat[:, 2 * g:2 * g + 2, :, :],
            fd[2 * g:2 * g + 2, :, :].rearrange("e (ch p) d -> p e ch d", p=128),
        )

    copy_alt = [0]

    def psum_copy(dst_ap, src_ap):
        """PSUM->SBUF copies ride the (less loaded) ACT engine."""
        nc.scalar.copy(dst_ap, src_ap)

    def pe_transpose_group(psum_view, srcs):
        """Transpose each [128|64,128] src into psum_view[:, k, :] via PE."""
        first = None
        for k, src in enumerate(srcs):
            mm = nc.tensor.matmul(
                psum_view[:, k, :], src, id_bf[0:src.shape[0], 0:src.shape[0]],
                is_transpose=True,
                start=(first is None), stop=(k == len(srcs) - 1),
                skip_group_check=True,
            )
            if first is None:
                first = mm
            else:
                add_dep_helper(mm.ins, first.ins, sync=False, reason="bank order")
        return first

    # ---- c^T via PE transposes: [128, EX, DH, C], p = d%128, free = i ----
    c_T = bigbuf.tile([128, EX, DH, C], BF16)
    for e in range(EX):
        for dh in range(DH):
            tp = ps_misc_pool.tile([128, CH, 128], BF16, tag="misc")
            pe_transpose_group(
                tp,
                [c_nat[:, e, chk, dh * 128:(dh + 1) * 128] for chk in range(CH)],
            )
            psum_copy(c_T[:, e, dh, :], tp[:, :, :])

    # ---- q^T via PE transposes: [128, EX, DH, Q], p = d%128, free = j ----
    q_T = bigbuf.tile([128, EX, DH, Q], BF16)
    for pair in range(EX // 2):
        tp = ps_misc_pool.tile([128, 4, Q], BF16, tag="misc")
        srcs = []
        for slot in range(2):
            for dh in range(DH):
                srcs.append(q_dup[0:64, 2 * pair + slot, dh * 128:(dh + 1) * 128])
        pe_transpose_group(tp, srcs)
        psum_copy(
            q_T[:, 2 * pair:2 * pair + 2, :, :],
            tp[:, :, :].rearrange("p (s dh) j -> p s dh j", s=2),
        )

    # ---- persistent per-core accumulators ----
    pm_col = sb_small.tile([128, EX * CH], BF16)    # Pmax^T (b_att numerators), col = e*CH+ch
    final_f = sb_small.tile([128, NK * EX], F32)    # col = (piece*DH+dh)*EX + e
    cmin_f = sb_small.tile([128, DH * EX], F32)     # col = dh*EX + e
    r_sb = sb_small.tile([128, EX], F32)            # 1/sum(pm), broadcast over partitions
    sumb = sb_small.tile([1, EX], F32)
    recipb = sb_small.tile([1, EX], F32)
    out_sb = sb_small.tile([EX, NL], F32)
    q2cT_sb = sb_small.tile([128, EX, DH], F32)

    fview = final_f[:, :].rearrange("p (pc dh e) -> p pc dh e", pc=4, dh=DH)
    cminv = cmin_f[:, :].rearrange("p (dh e) -> p dh e", dh=DH)

    dbg_tiles = []

    # Persistent phase intermediates (tiny vs 192KB/partition SBUF)
    P_all = sb_small.tile([128, CH, EX, Q], BF16)
    Pn_all = sb_small.tile([128, CH, EX, Q], BF16)
    PT_all = sb_small.tile([128, EX // 2, CH, 128], BF16)

    # ---------- phase A: S + softmax pieces, all examples ----------
    for e in range(EX):
        rhs_qm = qm_pool.tile([128, DH, Q], BF16)
        for dh in range(DH):
            nc.gpsimd.tensor_scalar(
                rhs_qm[:, dh, :], q_T[:, e, dh, :],
                w_sb[:, 4 + dh:5 + dh], w_sb[:, 0 + dh:1 + dh],
                op0=OP.mult, op1=OP.add,
            )
        ps_m = ps_misc_pool.tile([128, 512], F32, tag="misc")
        for dh in range(DH):
            nc.tensor.matmul(
                ps_m[0:1, 0:Q], wq_bf[:, dh:dh + 1], q_T[:, e, dh, :],
                start=(dh == 0), stop=(dh == DH - 1),
            )
        qwrow = qw_pool.tile([1, Q], BF16)
        nc.vector.tensor_copy(qwrow[0:1, :], ps_m[0:1, 0:Q])

        ps_s = ps_s_pool.tile([128, CH, Q], F32)
        first_mm = None
        for chk in range(CH):
            for dh in range(DH):
                mm = nc.tensor.matmul(
                    ps_s[:, chk, :],
                    c_T[:, e, dh, chk * 128:(chk + 1) * 128],
                    rhs_qm[:, dh, :],
                    start=(first_mm is None), stop=False,
                    skip_group_check=True,
                )
                if first_mm is None:
                    first_mm = mm
                else:
                    add_dep_helper(mm.ins, first_mm.ins, sync=False,
                                   reason="bank clear order")
        for chk in range(CH):
            mm = nc.tensor.matmul(
                ps_s[:, chk, :], ones_bf[0:1, :], qwrow[0:1, :],
                start=False, stop=(chk == CH - 1),
                skip_group_check=True,
            )
            add_dep_helper(mm.ins, first_mm.ins, sync=False,
                           reason="bank clear order")

        nc.scalar.activation(P_all[:, :, e, :], ps_s[:, :, :], AF.Exp)
        den = den_pool.tile([128, CH], F32)
        nc.vector.reduce_sum(den[:, :], P_all[:, :, e, :], axis=AX.X)
        nc.vector.tensor_reduce(
            pm_col[:, e * CH:(e + 1) * CH], P_all[:, :, e, :],
            axis=AX.X, op=OP.max,
        )
        rden = den_pool.tile([128, CH], F32, tag="rden")
        nc.vector.reciprocal(rden[:, :], den[:, :])
        nc.vector.tensor_tensor(
            Pn_all[:, :, e, :], P_all[:, :, e, :],
            rden[:, :].unsqueeze(2).broadcast_to([128, CH, Q]),
            op=OP.mult,
        )

    if DEBUG_DUMP:
        dbgP = sb_small.tile([128, CH * 2 * Q], BF16, tag="dbgP")
        nc.vector.tensor_copy(
            dbgP[:, :].rearrange("p (c s q) -> p c s q", c=CH, s=2),
            P_all[:, :, 0:2, :])
        dbg_tiles.append(("dbg_P", dbgP, [128, CH * 2 * Q], BF16))

    # ---------- phase B: P^T via PE + b_att normalizers, per pair ----------
    for pair in range(EX // 2):
        e0, e1 = 2 * pair, 2 * pair + 1
        tp = ps_misc_pool.tile([128, CH, 128], BF16, tag="misc")
        pe_transpose_group(tp, [Pn_all[:, chk, e0:e1 + 1, :] for chk in range(CH)])
        psum_copy(PT_all[:, pair, :, :], tp[:, :, :])

        ps_pair = ps_misc_pool.tile([128, 512], F32, tag="misc")
        nc.tensor.matmul(
            ps_pair[0:1, 0:2 * CH], ones128_bf[:, :],
            pm_col[:, e0 * CH:(e1 + 1) * CH],
            start=True, stop=True,
        )
        nc.vector.reduce_sum(
            sumb[0:1, e0:e1 + 1],
            ps_pair[0:1, 0:2 * CH].rearrange("o (e c) -> o e c", c=CH),
            axis=AX.X,
        )
        nc.vector.reciprocal(recipb[0:1, e0:e1 + 1], sumb[0:1, e0:e1 + 1])
        nc.tensor.matmul(
            ps_pair[:, 8:10], ones_f32[0:1, :], recipb[0:1, e0:e1 + 1],
            start=True, stop=True,
        )
        nc.vector.tensor_copy(r_sb[:, e0:e1 + 1], ps_pair[:, 8:10])

    # ---------- phase C: c2q + max-pools + q2c, all examples ----------
    for e in range(EX):
        pair, slot = e // 2, e % 2
        ps_c2q = ps_c2q_pool.tile([128, DH, C], F32)
        for dh in range(DH):
            nc.tensor.matmul(
                ps_c2q[:, dh, :],
                q_dup[slot * 64:slot * 64 + 64, e, dh * 128:(dh + 1) * 128],
                PT_all[slot * 64:slot * 64 + 64, pair, :, :],
                start=True, stop=True,
                tile_position=(slot * 64, 0),
            )
        c2q_sb = c2_pool.tile([128, DH, C], BF16)
        nc.scalar.copy(c2q_sb[:, :, :], ps_c2q[:, :, :])

        # piece 1: 2x-mode TT folds + short 1x reduce
        scrA = scr_pool.tile([128, DH, 256], BF16, tag="scrA")
        scrB = scr_pool.tile([128, DH, 128], BF16, tag="scrB")
        nc.vector.tensor_tensor(
            scrA[:, :, :], c2q_sb[:, :, 0:256], c2q_sb[:, :, 256:512], op=OP.max)
        nc.vector.tensor_tensor(
            scrB[:, :, :], scrA[:, :, 0:128], scrA[:, :, 128:256], op=OP.max)
        nc.vector.tensor_reduce(fview[:, 1, :, e], scrB[:, :, :], axis=AX.X, op=OP.max)

        # piece 2: max_i (c * c2q)
        prod = scr_pool.tile([128, DH, C], BF16, tag="prod")
        nc.vector.tensor_tensor(
            prod[:, :, :], c_T[:, e, :, :], c2q_sb[:, :, :], op=OP.mult)
        nc.vector.tensor_tensor(
            scrA[:, :, :], prod[:, :, 0:256], prod[:, :, 256:512], op=OP.max)
        nc.vector.tensor_tensor(
            scrB[:, :, :], scrA[:, :, 0:128], scrA[:, :, 128:256], op=OP.max)
        nc.vector.tensor_reduce(fview[:, 2, :, e], scrB[:, :, :], axis=AX.X, op=OP.max)

        # q2c = b_att @ c (unnormalized weights pm, scaled by 1/sum)
        ps_m2 = ps_misc_pool.tile([128, 512], F32, tag="misc")
        for chk in range(CH):
            nc.tensor.matmul(
                ps_m2[0:1, 64:64 + H2],
                pm_col[:, e * CH + chk:e * CH + chk + 1],
                c_nat[:, e, chk, :],
                start=(chk == 0), stop=(chk == CH - 1),
            )
        q2c_sc = q2_pool.tile([1, H2], F32)
        nc.scalar.mul(q2c_sc[0:1, :], ps_m2[0:1, 64:64 + H2], r_sb[0:1, e:e + 1])
        for dh in range(DH):
            nc.tensor.matmul(
                ps_m2[:, 320 + dh:321 + dh],
                q2c_sc[0:1, dh * 128:(dh + 1) * 128],
                ones_f32[0:1, 0:1],
                is_transpose=True,
                start=(dh == 0), stop=(dh == DH - 1),
                skip_group_check=True,
            )
        nc.scalar.copy(q2cT_sb[:, e, :], ps_m2[:, 320:322])

    # ---- piece 0 (max_i c, min_i c) for all examples: batched folds ----
    cT_all = c_T[:, :, :, :]                        # [128, EX, DH, C]
    for op, dst in ((OP.max, fview[:, 0, :, :].rearrange("p dh e -> p e dh")),
                    (OP.min, cminv[:, :, :].rearrange("p dh e -> p e dh"))):
        f1 = scr_pool.tile([128, EX, DH, 256], BF16, tag="bigA")
        nc.vector.tensor_tensor(
            f1[:, :, :, :], cT_all[:, :, :, 0:256], cT_all[:, :, :, 256:512], op=op)
        f2 = scr_pool.tile([128, EX, DH, 128], BF16, tag="bigB")
        nc.vector.tensor_tensor(
            f2[:, :, :, :], f1[:, :, :, 0:128], f1[:, :, :, 128:256], op=op)
        f3 = scr_pool.tile([128, EX, DH, 64], BF16, tag="bigC")
        nc.vector.tensor_tensor(
            f3[:, :, :, :], f2[:, :, :, 0:64], f2[:, :, :, 64:128], op=op)
        nc.vector.tensor_reduce(dst, f3[:, :, :, :], axis=AX.X, op=op)

    # ---- piece 3 batched: max(q2c*cmax, q2c*cmin) over all examples ----
    s3a = s3_pool.tile([128, EX, DH], F32, tag="s3a")
    s3b = s3_pool.tile([128, EX, DH], F32, tag="s3b")
    nc.vector.tensor_tensor(
        s3a[:, :, :], q2cT_sb[:, :, :],
        fview[:, 0, :, :].rearrange("p dh e -> p e dh"), op=OP.mult)
    nc.vector.tensor_tensor(
        s3b[:, :, :], q2cT_sb[:, :, :],
        cminv[:, :, :].rearrange("p dh e -> p e dh"), op=OP.mult)
    nc.vector.tensor_tensor(
        fview[:, 3, :, :].rearrange("p dh e -> p e dh"), s3a[:, :, :], s3b[:, :, :],
        op=OP.max)

    # ---------- final: out = max-pooled features @ w_label + b ----------
    ps_out = ps_misc_pool.tile([128, 512], F32, tag="misc")
    for k in range(NK):
        nc.tensor.matmul(
            ps_out[0:EX, 0:NL], final_f[:, k * EX:(k + 1) * EX], wlab_sb[:, k, :],
            start=(k == 0), stop=False, skip_group_check=True,
        )
    nc.tensor.matmul(
        ps_out[0:EX, 0:NL], ones_f32[0:1, 0:EX], b_sb[0:1, :],
        start=False, stop=True, skip_group_check=True,
    )
    nc.vector.tensor_copy(out_sb[:, :], ps_out[0:EX, 0:NL])
    nc.sync.dma_start(out[:, :], out_sb[:, :])

    if DEBUG_DUMP:
        for name, tl, shp, dt in [
            ("dbg_final", final_f, [128, NK * EX], F32),
            ("dbg_cmin", cmin_f, [128, DH * EX], F32),
            ("dbg_pm", pm_col, [128, EX * CH], BF16),
            ("dbg_r", r_sb, [128, EX], F32),
            ("dbg_sumb", sumb, [1, EX], F32),
        ] + dbg_tiles:
            t = nc.dram_tensor(name, shp, dt, kind="ExternalOutput")
            nc.sync.dma_start(t[:, :], tl[:, :])


def build_nc():
    nc = bacc.Bacc("TRN2", target_bir_lowering=False, debug=False)
    fd = nc.dram_tensor("fd", [EX, C, H2], F32, kind="ExternalInput")
    fq = nc.dram_tensor("fq", [EX, Q, H2], F32, kind="ExternalInput")
    wsim = nc.dram_tensor("wsim", [3 * H2], F32, kind="ExternalInput")
    wlab = nc.dram_tensor("wlab", [4 * H2, NL], F32, kind="ExternalInput")
    blab = nc.dram_tensor("blab", [NL], F32, kind="ExternalInput")
    out = nc.dram_tensor("out", [EX, NL], F32, kind="ExternalOutput")

    from contextlib import ExitStack
    with tile.TileContext(nc) as tc:
        with ExitStack() as ctx:
            _body(tc, ctx, fd[:, :, :], fq[:, :, :], wsim[:], wlab[:, :], blab[:], out[:, :])
    nc.compile()
    return nc


_NC_CACHE = None


def run(inputs, trace=False):
    global _NC_CACHE
    if _NC_CACHE is None:
        _NC_CACHE = build_nc()
    nc = _NC_CACHE

    fd = np.ascontiguousarray(np.asarray(inputs["feature_document"], dtype=np.float32))
    fq = np.ascontiguousarray(np.asarray(inputs["feature_query"], dtype=np.float32))
    wsim = np.ascontiguousarray(np.asarray(inputs["w_sim"], dtype=np.float32))
    wlab = np.ascontiguousarray(np.asarray(inputs["w_label"], dtype=np.float32))
    blab = np.ascontiguousarray(np.asarray(inputs["b_label"], dtype=np.float32))

    in_maps = []
    for core in range(N_CORES):
        sl = slice(core * EX, (core + 1) * EX)
        in_maps.append({
            "fd": fd[sl], "fq": fq[sl],
            "wsim": wsim, "wlab": wlab, "blab": blab,
        })
    res = run_bass_kernel_spmd(nc, in_maps, list(range(N_CORES)), trace=trace)
    outs = np.concatenate([np.asarray(res.results[i]["out"]) for i in range(N_CORES)], axis=0)
    return outs.astype(np.float32), res


def kernel(**inputs):
    outs, _ = run(inputs, trace=False)
    return outs
